# revision 9
# baseline (speedup 1.0000x reference)
# Llama attention layer (B=1, T=4096, D=2048, 16 heads) on 8 TRN2 NeuronCores.
#
# Sharding: tensor-parallel over heads. Each core computes 2 heads:
#   - Wq/Wk/Wv sharded column-wise (rows of the [out,in] weight), Wo row-wise.
#   - Each core produces a partial [T, D] o_proj output; the host sums the 8
#     partials (the "all-reduce" of the hint, done on the host since the
#     contract is full-in/full-out).
#
# v3 layout/schedule:
#   - x is read from HBM ONCE per t-tile j, cast f32->bf16 in the DMA (SWDGE),
#     in 4 chunk-tiles so consumers wait at 1 MB granularity.
#   - Everything bf16 except psum accumulators (PE rate identical, DVE 2x,
#     SBUF/DMA halved, FWL fast weight loads).
#   - Q/K produced per head-PAIR in one [128, 1024] psum tile (2 banks); rope
#     runs on the pair (1 ACT copy + 2 swap DMAs + 3 DVE TTs per pair).
#   - Attention processes k-tiles in PAIRS: scores for (kt, kt+1) land in one
#     [128, 1024] psum tile -> ONE exp ACTIVATE per pair. Causal masking via
#     two baked pair-mask tables, one DVE mul per diagonal pair.
#   - 3-deep software pipeline across t-tiles: loop j emits proj(j+2),
#     attention(j) and oproj(j-1) interleaved at unit granularity, so the PE
#     stream stays dense (HAM warm) and the serial softmax-tail chains
#     (gpsimd partition reduce -> reciprocal -> normalize) hide under
#     independent projection matmuls from two t-tiles ahead.
#   - exp without max-subtraction (|logits| <= ~6, exact in fp32 psum).

import sys

import numpy as np

for _p in ("/opt/trn_rl_repo",):
    if _p not in sys.path:
        sys.path.insert(0, _p)

import ml_dtypes  # noqa: E402

import concourse.bass as bass  # noqa: E402
from concourse import bacc  # noqa: E402
import concourse.tile as tile  # noqa: E402
from concourse import bass_isa, bass_utils, mybir  # noqa: E402

B, T, D = 1, 4096, 2048
NH, HD = 16, 128
NCORES = 8
HPC = NH // NCORES  # heads per core = 2
DCORE = HPC * HD  # 256
P = 128
TT = 512  # t/q tile (free dim)
NT = T // TT  # 8
NCT = D // P  # 16 contraction tiles for the projections
ROPE_BASE = 10000.0
SCALE = 1.0 / float(np.sqrt(HD))

F32 = mybir.dt.float32
BF16 = mybir.dt.bfloat16
DEBUG = False


def _emit(nc, tc, h):
    import contextlib

    ctx = contextlib.ExitStack()
    with ctx:
        const = ctx.enter_context(tc.tile_pool(name="const", bufs=1))
        xjp = ctx.enter_context(tc.tile_pool(name="xj", bufs=12))
        csp = ctx.enter_context(tc.tile_pool(name="cs", bufs=4))
        rp = ctx.enter_context(tc.tile_pool(name="rope", bufs=5))
        qp = ctx.enter_context(tc.tile_pool(name="qq", bufs=4))
        kkp = ctx.enter_context(tc.tile_pool(name="kk", bufs=NT))
        vp = ctx.enter_context(tc.tile_pool(name="v", bufs=1))
        ptp = ctx.enter_context(tc.tile_pool(name="pt", bufs=3))
        lap = ctx.enter_context(tc.tile_pool(name="lacc", bufs=3))
        lrp = ctx.enter_context(tc.tile_pool(name="lrep", bufs=2))
        ryp = ctx.enter_context(tc.tile_pool(name="ry", bufs=2))
        ytp = ctx.enter_context(tc.tile_pool(name="yt", bufs=8))
        obp = ctx.enter_context(tc.tile_pool(name="ob", bufs=2))

        # ---- persistent tiles ------------------------------------------------
        wq_sb = const.tile([P, NCT, DCORE], BF16, tag="wq")
        wk_sb = const.tile([P, NCT, DCORE], BF16, tag="wk")
        wv_sb = const.tile([P, NCT, DCORE], BF16, tag="wv")
        wo_sb = const.tile([P, HPC, D], BF16, tag="wo")
        maskp = const.tile([P, 2, 2 * TT], BF16, tag="maskp")
        ones_col = const.tile([P, 1], BF16, tag="ones_col")
        ones_row = const.tile([1, P], F32, tag="ones_row")
        nc.vector.memset(ones_col[:], 1.0)
        nc.vector.memset(ones_row[:], 1.0)

        # chunked weight loads: first matmuls only wait on their c-chunk
        for name, wsb in (("wq", wq_sb), ("wk", wk_sb), ("wv", wv_sb)):
            wr = h[name].rearrange("(co ci) d -> ci co d", ci=P)
            for c0 in range(0, NCT, 4):
                nc.sync.dma_start(wsb[:, c0 : c0 + 4, :], wr[:, c0 : c0 + 4, :])
        nc.sync.dma_start(wo_sb[:], h["wo"].rearrange("(ds di) e -> di ds e", di=P))
        nc.sync.dma_start(maskp[:], h["maskp"].rearrange("p (dp q) -> p dp q", dp=2))

        xt_r = h["xt"].rearrange("(c p) t -> p c t", p=P)

        ks = [None] * NT  # [128, 2*TT] bf16 per j: [:, hh*TT:(hh+1)*TT] = head hh
        qs = [None] * NT
        yts = [[None] * NT for _ in range(HPC)]
        v_sb = vp.tile([P, T // P, DCORE], BF16, tag="v")

        with tc.tile_pool(name="pp", bufs=4, space="PSUM") as pp:
            xjs = [None] * NT
            css = [None] * NT

            def load_xj(j):
                # SWDGE cast f32 -> bf16 during the transfer; 4 chunk tiles of
                # 1 MB (src) pipeline Q7 emission and give fine-grained deps.
                chunks = []
                for ci in range(4):
                    xc = xjp.tile([P, 4, TT], BF16, tag="xjc", name=f"xj{j}_{ci}")
                    nc.gpsimd.dma_start(
                        xc[:],
                        xt_r[:, 4 * ci : 4 * ci + 4, j * TT : (j + 1) * TT],
                    )
                    chunks.append(xc)
                xjs[j] = chunks
                cs2 = csp.tile([P, 2, 2, TT], BF16, tag="cs", name=f"cs{j}")
                for hh in range(HPC):
                    nc.sync.dma_start(
                        cs2[:, 0, hh, :], h["cos"][:, j * TT : (j + 1) * TT]
                    )
                    nc.sync.dma_start(
                        cs2[:, 1, hh, :], h["sin"][:, j * TT : (j + 1) * TT]
                    )
                css[j] = cs2

            def proj_chunk(j, w_sb, ps, c0):
                # one of 4 chunks of the Q (or K) projection wave: 8 MMs
                xc = xjs[j][c0 // 4]
                for ci in range(4):
                    c = c0 + ci
                    for hh in range(HPC):
                        nc.tensor.matmul(
                            ps[:, hh * TT : (hh + 1) * TT],
                            w_sb[:, c, hh * HD : (hh + 1) * HD],
                            xc[:, ci, :],
                            start=(c == 0),
                            stop=(c == NCT - 1),
                        )

            def rope(j, ps, dest_arr, dpool, dtag):
                cs2 = css[j]
                raw = rp.tile([P, 2 * TT], BF16, tag="rp")
                nc.scalar.copy(raw[:], ps[:])
                sw = rp.tile([P, 2 * TT], BF16, tag="rp")
                nc.gpsimd.dma_start(sw[0:64, :], raw[64:128, :])
                nc.gpsimd.dma_start(sw[64:128, :], raw[0:64, :])
                qc = rp.tile([P, 2 * TT], BF16, tag="rp")
                nc.vector.tensor_mul(qc[:], raw[:], cs2[:, 0, :, :])
                nc.vector.tensor_mul(sw[:], sw[:], cs2[:, 1, :, :])
                dest = dpool.tile([P, 2 * TT], BF16, tag=dtag)
                nc.vector.tensor_add(dest[:], qc[:], sw[:])
                dest_arr[j] = dest

            def proj_v(j, half):
                # half 0: s=0,1 ; half 1: s=2,3. 32 MMs into one [128,1024] tile
                psv = pp.tile([P, 2 * TT], F32, tag="ps", name=f"psv{j}_{half}")
                for c in range(NCT):
                    xc = xjs[j][c // 4]
                    for s2 in range(2):
                        s = 2 * half + s2
                        nc.tensor.matmul(
                            psv[:, s2 * TT : s2 * TT + DCORE],
                            xc[:, c % 4, s * P : (s + 1) * P],
                            wv_sb[:, c, :],
                            start=(c == 0),
                            stop=(c == NCT - 1),
                        )
                for s2 in range(2):
                    s = 2 * half + s2
                    nc.scalar.copy(
                        v_sb[:, 4 * j + s, :], psv[:, s2 * TT : s2 * TT + DCORE]
                    )

            def attn_unit(j, hh, kp, psy, lacc, nkp):
                # one k-pair of attention for q-tile j, head hh
                pss = pp.tile([P, 2 * TT], F32, tag="ps", name=f"pss{j}_{hh}_{kp}")
                qr = qs[j][:, hh * TT : (hh + 1) * TT]
                for half in range(2):
                    kt = 2 * kp + half
                    lhsT = ks[kt // 4][
                        :, hh * TT + (kt % 4) * P : hh * TT + (kt % 4 + 1) * P
                    ]
                    nc.tensor.matmul(
                        pss[:, half * TT : (half + 1) * TT], lhsT, qr,
                        start=True, stop=True,
                    )
                pt = ptp.tile([P, 2 * TT], BF16, tag="pt")
                nc.scalar.activation(
                    pt[:], pss[:], mybir.ActivationFunctionType.Exp, scale=SCALE
                )
                dp = kp - 2 * j  # diagonal pair index (0 or 1) if >= 0
                if dp >= 0:
                    nc.vector.tensor_mul(pt[:], pt[:], maskp[:, dp, :])
                for half in range(2):
                    kt = 2 * kp + half
                    nc.tensor.matmul(
                        psy[:, hh * TT : (hh + 1) * TT],
                        v_sb[:, kt, hh * HD : (hh + 1) * HD],
                        pt[:, half * TT : (half + 1) * TT],
                        start=(kp == 0 and half == 0),
                        stop=(kp == nkp - 1 and half == 1),
                    )
                if kp == 0:
                    nc.vector.tensor_copy(lacc[:], pt[:, 0:TT])
                else:
                    nc.vector.tensor_add(lacc[:], lacc[:], pt[:, 0:TT])
                nc.vector.tensor_add(lacc[:], lacc[:], pt[:, TT : 2 * TT])

            def attn_tail(j, hh, psy, lacc):
                # denominator: ones-matmul partition-sum -> reciprocal ->
                # PE broadcast -> normalize. All fast units, no gpsimd.
                dps = pp.tile([P, 2 * TT], F32, tag="ps", name=f"dps{j}_{hh}")
                nc.tensor.matmul(
                    dps[0:1, 0:TT], ones_col[:], lacc[:], start=True, stop=True
                )
                rinv1 = ryp.tile([1, TT], F32, tag="rinv")
                nc.vector.reciprocal_approx_fast(rinv1[:], dps[0:1, 0:TT])
                nc.tensor.matmul(
                    dps[:, TT : 2 * TT], ones_row[:], rinv1[:],
                    start=True, stop=True,
                )
                rb = lrp.tile([P, TT], BF16, tag="lrep")
                nc.scalar.copy(rb[:], dps[:, TT : 2 * TT])
                yt = ytp.tile([P, TT], BF16, tag="yt")
                nc.vector.tensor_mul(yt[:], psy[:, hh * TT : (hh + 1) * TT], rb[:])
                yts[hh][j] = yt

            def oproj_unit(jj, s):
                # output rows t0..t0+127 ; 2 psum banks at a time (4 e-halves)
                ob = obp.tile([P, D], BF16, tag="ob")
                for eh in range(2):
                    pso = pp.tile([P, 2 * TT], F32, tag="ps", name=f"pso{jj}_{s}_{eh}")
                    for e2 in range(2):
                        e = 2 * eh + e2
                        for hh in range(HPC):
                            nc.tensor.matmul(
                                pso[:, e2 * TT : (e2 + 1) * TT],
                                yts[hh][jj][:, s * P : (s + 1) * P],
                                wo_sb[:, hh, e * TT : (e + 1) * TT],
                                start=(hh == 0),
                                stop=(hh == HPC - 1),
                            )
                    if eh == 0:
                        nc.vector.tensor_copy(ob[:, 0 : 2 * TT], pso[:])
                    else:
                        nc.scalar.copy(ob[:, 2 * TT : 4 * TT], pso[:])
                t0 = jj * TT + s * P
                nc.sync.dma_start(h["out"][t0 : t0 + P, :], ob[:])

            def make_proj_units(j):
                units = []
                psq = pp.tile([P, 2 * TT], F32, tag="ps", name=f"psq{j}")
                for c0 in (0, 4, 8, 12):
                    units.append(
                        lambda j=j, psq=psq, c0=c0: proj_chunk(j, wq_sb, psq, c0)
                    )
                units.append(lambda j=j, psq=psq: rope(j, psq, qs, qp, "qq"))
                psk = pp.tile([P, 2 * TT], F32, tag="ps", name=f"psk{j}")
                for c0 in (0, 4, 8, 12):
                    units.append(
                        lambda j=j, psk=psk, c0=c0: proj_chunk(j, wk_sb, psk, c0)
                    )
                units.append(lambda j=j, psk=psk: rope(j, psk, ks, kkp, "kk"))
                for half in range(2):
                    units.append(lambda j=j, half=half: proj_v(j, half))
                return units

            # ---- pipelined emission -----------------------------------------
            # prologue: x for t-tiles 0-1, projections for t-tile 0
            load_xj(0)
            load_xj(1)
            for u in make_proj_units(0):
                u()

            for j in range(NT):
                if j + 2 < NT:
                    load_xj(j + 2)

                proj_units = make_proj_units(j + 1) if j + 1 < NT else []

                att_units = []
                nkp = 2 * j + 2
                psy = pp.tile([P, 2 * TT], F32, tag="ps", name=f"psy{j}")
                laccs = [
                    lap.tile([P, TT], BF16, tag="lacc", name=f"lacc{j}_{_h}")
                    for _h in range(HPC)
                ]
                for kp in range(nkp):
                    for hh in range(HPC):
                        att_units.append(
                            lambda j=j, hh=hh, kp=kp, psy=psy, l=laccs, n=nkp:
                            attn_unit(j, hh, kp, psy, l[hh], n)
                        )
                for hh in range(HPC):
                    att_units.append(
                        lambda j=j, hh=hh, psy=psy, l=laccs:
                        attn_tail(j, hh, psy, l[hh])
                    )
                if j >= 2:
                    for s in range(4):
                        att_units.append(lambda jj=j - 2, s=s: oproj_unit(jj, s))

                # Round-robin the two streams so the PE queue stays dense.
                na, np_ = len(att_units), len(proj_units)
                ia = ip = 0
                while ia < na or ip < np_:
                    if ip * max(na, 1) <= ia * max(np_, 1):
                        if ip < np_:
                            proj_units[ip]()
                            ip += 1
                        else:
                            att_units[ia]()
                            ia += 1
                    else:
                        if ia < na:
                            att_units[ia]()
                            ia += 1
                        else:
                            proj_units[ip]()
                            ip += 1

            for jj in (NT - 2, NT - 1):
                for s in range(4):
                    oproj_unit(jj, s)

        if DEBUG:
            nc.sync.dma_start(h["dbg_q"][:], qs[0][:, 0:TT])
            nc.sync.dma_start(h["dbg_k"][:], ks[0][:, 0:TT])
            nc.sync.dma_start(h["dbg_v"][:], v_sb[:, 0, :])
            nc.sync.dma_start(h["dbg_y"][:], yts[0][0][:])


_CACHE = {}


def _program():
    if "nc" in _CACHE:
        return _CACHE["nc"]
    nc = bacc.Bacc(trn_type="TRN2")
    h = {
        "xt": nc.dram_tensor("xt", [D, T], F32, kind="ExternalInput"),
        "wq": nc.dram_tensor("wq", [D, DCORE], BF16, kind="ExternalInput"),
        "wk": nc.dram_tensor("wk", [D, DCORE], BF16, kind="ExternalInput"),
        "wv": nc.dram_tensor("wv", [D, DCORE], BF16, kind="ExternalInput"),
        "wo": nc.dram_tensor("wo", [DCORE, D], BF16, kind="ExternalInput"),
        "cos": nc.dram_tensor("cos", [P, T], BF16, kind="ExternalInput"),
        "sin": nc.dram_tensor("sin", [P, T], BF16, kind="ExternalInput"),
        "maskp": nc.dram_tensor("maskp", [P, 2 * 2 * TT], BF16, kind="ExternalInput"),
        "out": nc.dram_tensor("out", [T, D], BF16, kind="ExternalOutput"),
    }
    if DEBUG:
        h["dbg_q"] = nc.dram_tensor("dbg_q", [P, TT], BF16, kind="ExternalOutput")
        h["dbg_k"] = nc.dram_tensor("dbg_k", [P, TT], BF16, kind="ExternalOutput")
        h["dbg_v"] = nc.dram_tensor("dbg_v", [P, DCORE], BF16, kind="ExternalOutput")
        h["dbg_y"] = nc.dram_tensor("dbg_y", [P, TT], BF16, kind="ExternalOutput")
    with tile.TileContext(nc) as tc:
        _emit(nc, tc, h)
    nc.compile()
    _CACHE["nc"] = nc
    return nc


def _bf16(a):
    return np.asarray(a, dtype=np.float32).astype(ml_dtypes.bfloat16)


def _host_inputs(x, Wq, Wk, Wv, Wo):
    x = np.asarray(x, dtype=np.float32)
    xT = np.ascontiguousarray(x.reshape(T, D).T)  # [D, T]

    # rope tables, de-interleaved (evens then odds) with sign baked into sin
    inv = 1.0 / (ROPE_BASE ** (np.arange(0, HD, 2, dtype=np.float32) / HD))
    t = np.arange(T, dtype=np.float32)
    freqs = t[:, None] * inv[None, :]  # [T, 64]
    emb = np.concatenate([freqs, freqs], axis=-1)  # [T, 128]
    cos = np.cos(emb)
    sin = np.sin(emb)
    perm = np.concatenate([np.arange(0, HD, 2), np.arange(1, HD, 2)])
    cos_d = np.ascontiguousarray(cos[:, perm].T)  # [128, T]
    sgn = np.concatenate([-np.ones(64), np.ones(64)]).astype(np.float32)
    sin_d = np.ascontiguousarray(sgn[:, None] * sin[:, perm].T)

    # pair masks: maskp[k, dp*1024 + h*512 + q] = 1 iff q >= k + (2*dp+h)*128
    kk = np.arange(P)[:, None]
    qq = np.arange(TT)[None, :]
    mp = np.zeros((P, 2, 2, TT), dtype=np.float32)
    for dp in range(2):
        for hf in range(2):
            mp[:, dp, hf, :] = (qq >= kk + (2 * dp + hf) * P).astype(np.float32)
    mp = mp.reshape(P, 2 * 2 * TT)

    maps = []
    for i in range(NCORES):
        rows = np.concatenate(
            [(2 * i + hh) * HD + perm for hh in range(HPC)]
        )  # de-interleaved q/k rows for this core's heads
        vrows = np.arange(i * DCORE, (i + 1) * DCORE)
        maps.append(
            {
                "xt": xT,
                "wq": _bf16(np.asarray(Wq, np.float32)[rows, :].T),
                "wk": _bf16(np.asarray(Wk, np.float32)[rows, :].T),
                "wv": _bf16(np.asarray(Wv, np.float32)[vrows, :].T),
                "wo": _bf16(np.asarray(Wo, np.float32)[:, vrows].T),
                "cos": _bf16(cos_d),
                "sin": _bf16(sin_d),
                "maskp": _bf16(mp),
            }
        )
    return maps


def _run(x, Wq, Wk, Wv, Wo, trace=False):
    nc = _program()
    maps = _host_inputs(x, Wq, Wk, Wv, Wo)
    kw = {}
    if trace:
        kw = {"trace": True, "trace_cores": [0]}
    res = bass_utils.run_bass_kernel_spmd(
        nc, maps, core_ids=list(range(NCORES)), **kw
    )
    acc = np.zeros((T, D), dtype=np.float32)
    for r in res.results:
        acc += np.asarray(r["out"]).astype(np.float32)
    return acc.reshape(B, T, D), res


def kernel(x, Wq, Wk, Wv, Wo):
    out, _ = _run(x, Wq, Wk, Wv, Wo, trace=False)
    return out


# revision 10
# speedup vs baseline: 1.0217x; 1.0217x over previous
# Llama attention layer (B=1, T=4096, D=2048, 16 heads) on 8 TRN2 NeuronCores.
#
# Sharding: tensor-parallel over heads. Each core computes 2 heads:
#   - Wq/Wk/Wv sharded column-wise (rows of the [out,in] weight), Wo row-wise.
#   - Each core produces a partial [T, D] o_proj output; the host sums the 8
#     partials (the "all-reduce" of the hint, done on the host since the
#     contract is full-in/full-out).
#
# v3 layout/schedule:
#   - x is read from HBM ONCE per t-tile j, cast f32->bf16 in the DMA (SWDGE),
#     in 4 chunk-tiles so consumers wait at 1 MB granularity.
#   - Everything bf16 except psum accumulators (PE rate identical, DVE 2x,
#     SBUF/DMA halved, FWL fast weight loads).
#   - Q/K produced per head-PAIR in one [128, 1024] psum tile (2 banks); rope
#     runs on the pair (1 ACT copy + 2 swap DMAs + 3 DVE TTs per pair).
#   - Attention processes k-tiles in PAIRS: scores for (kt, kt+1) land in one
#     [128, 1024] psum tile -> ONE exp ACTIVATE per pair. Causal masking via
#     two baked pair-mask tables, one DVE mul per diagonal pair.
#   - 3-deep software pipeline across t-tiles: loop j emits proj(j+2),
#     attention(j) and oproj(j-1) interleaved at unit granularity, so the PE
#     stream stays dense (HAM warm) and the serial softmax-tail chains
#     (gpsimd partition reduce -> reciprocal -> normalize) hide under
#     independent projection matmuls from two t-tiles ahead.
#   - exp without max-subtraction (|logits| <= ~6, exact in fp32 psum).

import sys

import numpy as np

for _p in ("/opt/trn_rl_repo",):
    if _p not in sys.path:
        sys.path.insert(0, _p)

import ml_dtypes  # noqa: E402

import concourse.bass as bass  # noqa: E402
from concourse import bacc  # noqa: E402
import concourse.tile as tile  # noqa: E402
from concourse import bass_isa, bass_utils, mybir  # noqa: E402

B, T, D = 1, 4096, 2048
NH, HD = 16, 128
NCORES = 8
HPC = NH // NCORES  # heads per core = 2
DCORE = HPC * HD  # 256
P = 128
TT = 512  # t/q tile (free dim)
NT = T // TT  # 8
NCT = D // P  # 16 contraction tiles for the projections
ROPE_BASE = 10000.0
SCALE = 1.0 / float(np.sqrt(HD))

F32 = mybir.dt.float32
BF16 = mybir.dt.bfloat16
DEBUG = False


def _emit(nc, tc, h):
    import contextlib

    ctx = contextlib.ExitStack()
    with ctx:
        const = ctx.enter_context(tc.tile_pool(name="const", bufs=1))
        xjp = ctx.enter_context(tc.tile_pool(name="xj", bufs=12))
        csp = ctx.enter_context(tc.tile_pool(name="cs", bufs=4))
        rp = ctx.enter_context(tc.tile_pool(name="rope", bufs=5))
        qp = ctx.enter_context(tc.tile_pool(name="qq", bufs=4))
        kkp = ctx.enter_context(tc.tile_pool(name="kk", bufs=NT))
        vp = ctx.enter_context(tc.tile_pool(name="v", bufs=1))
        ptp = ctx.enter_context(tc.tile_pool(name="pt", bufs=3))
        lap = ctx.enter_context(tc.tile_pool(name="lacc", bufs=3))
        lrp = ctx.enter_context(tc.tile_pool(name="lrep", bufs=2))
        ryp = ctx.enter_context(tc.tile_pool(name="ry", bufs=2))
        ytp = ctx.enter_context(tc.tile_pool(name="yt", bufs=8))
        obp = ctx.enter_context(tc.tile_pool(name="ob", bufs=2))

        # ---- persistent tiles ------------------------------------------------
        wq_sb = const.tile([P, NCT, DCORE], BF16, tag="wq")
        wk_sb = const.tile([P, NCT, DCORE], BF16, tag="wk")
        wv_sb = const.tile([P, NCT, DCORE], BF16, tag="wv")
        wo_sb = const.tile([P, HPC, D], BF16, tag="wo")
        maskp = const.tile([P, 2, 2 * TT], BF16, tag="maskp")
        ones_col = const.tile([P, 1], BF16, tag="ones_col")
        ones_row = const.tile([1, P], BF16, tag="ones_row")
        nc.vector.memset(ones_col[:], 1.0)
        nc.vector.memset(ones_row[:], 1.0)

        # chunked weight loads: first matmuls only wait on their c-chunk
        for name, wsb in (("wq", wq_sb), ("wk", wk_sb), ("wv", wv_sb)):
            wr = h[name].rearrange("(co ci) d -> ci co d", ci=P)
            for c0 in range(0, NCT, 4):
                nc.sync.dma_start(wsb[:, c0 : c0 + 4, :], wr[:, c0 : c0 + 4, :])
        nc.sync.dma_start(wo_sb[:], h["wo"].rearrange("(ds di) e -> di ds e", di=P))
        nc.sync.dma_start(maskp[:], h["maskp"].rearrange("p (dp q) -> p dp q", dp=2))

        xt_r = h["xt"].rearrange("(c p) t -> p c t", p=P)

        ks = [None] * NT  # [128, 2*TT] bf16 per j: [:, hh*TT:(hh+1)*TT] = head hh
        qs = [None] * NT
        yts = [[None] * NT for _ in range(HPC)]
        v_sb = vp.tile([P, T // P, DCORE], BF16, tag="v")

        with tc.tile_pool(name="pp", bufs=4, space="PSUM") as pp:
            xjs = [None] * NT
            css = [None] * NT

            def load_xj(j):
                # SWDGE cast f32 -> bf16 during the transfer; 4 chunk tiles of
                # 1 MB (src) pipeline Q7 emission and give fine-grained deps.
                chunks = []
                for ci in range(4):
                    xc = xjp.tile([P, 4, TT], BF16, tag="xjc", name=f"xj{j}_{ci}")
                    nc.gpsimd.dma_start(
                        xc[:],
                        xt_r[:, 4 * ci : 4 * ci + 4, j * TT : (j + 1) * TT],
                    )
                    chunks.append(xc)
                xjs[j] = chunks
                cs2 = csp.tile([P, 2, 2, TT], BF16, tag="cs", name=f"cs{j}")
                for hh in range(HPC):
                    nc.sync.dma_start(
                        cs2[:, 0, hh, :], h["cos"][:, j * TT : (j + 1) * TT]
                    )
                    nc.sync.dma_start(
                        cs2[:, 1, hh, :], h["sin"][:, j * TT : (j + 1) * TT]
                    )
                css[j] = cs2

            def proj_chunk(j, w_sb, ps, c0):
                # one of 4 chunks of the Q (or K) projection wave: 8 MMs
                xc = xjs[j][c0 // 4]
                for ci in range(4):
                    c = c0 + ci
                    for hh in range(HPC):
                        nc.tensor.matmul(
                            ps[:, hh * TT : (hh + 1) * TT],
                            w_sb[:, c, hh * HD : (hh + 1) * HD],
                            xc[:, ci, :],
                            start=(c == 0),
                            stop=(c == NCT - 1),
                        )

            def rope(j, ps, dest_arr, dpool, dtag):
                cs2 = css[j]
                raw = rp.tile([P, 2 * TT], BF16, tag="rp")
                nc.scalar.copy(raw[:], ps[:])
                sw = rp.tile([P, 2 * TT], BF16, tag="rp")
                nc.gpsimd.dma_start(sw[0:64, :], raw[64:128, :])
                nc.gpsimd.dma_start(sw[64:128, :], raw[0:64, :])
                qc = rp.tile([P, 2 * TT], BF16, tag="rp")
                nc.vector.tensor_mul(qc[:], raw[:], cs2[:, 0, :, :])
                nc.vector.tensor_mul(sw[:], sw[:], cs2[:, 1, :, :])
                dest = dpool.tile([P, 2 * TT], BF16, tag=dtag)
                nc.vector.tensor_add(dest[:], qc[:], sw[:])
                dest_arr[j] = dest

            def proj_v(j, half):
                # half 0: s=0,1 ; half 1: s=2,3. 32 MMs into one [128,1024] tile
                psv = pp.tile([P, 2 * TT], F32, tag="ps", name=f"psv{j}_{half}")
                for c in range(NCT):
                    xc = xjs[j][c // 4]
                    for s2 in range(2):
                        s = 2 * half + s2
                        nc.tensor.matmul(
                            psv[:, s2 * TT : s2 * TT + DCORE],
                            xc[:, c % 4, s * P : (s + 1) * P],
                            wv_sb[:, c, :],
                            start=(c == 0),
                            stop=(c == NCT - 1),
                        )
                for s2 in range(2):
                    s = 2 * half + s2
                    nc.scalar.copy(
                        v_sb[:, 4 * j + s, :], psv[:, s2 * TT : s2 * TT + DCORE]
                    )

            def attn_unit(j, hh, kp, psy, lacc, nkp):
                # one k-pair of attention for q-tile j, head hh
                pss = pp.tile([P, 2 * TT], F32, tag="ps", name=f"pss{j}_{hh}_{kp}")
                qr = qs[j][:, hh * TT : (hh + 1) * TT]
                for half in range(2):
                    kt = 2 * kp + half
                    lhsT = ks[kt // 4][
                        :, hh * TT + (kt % 4) * P : hh * TT + (kt % 4 + 1) * P
                    ]
                    nc.tensor.matmul(
                        pss[:, half * TT : (half + 1) * TT], lhsT, qr,
                        start=True, stop=True,
                    )
                pt = ptp.tile([P, 2 * TT], BF16, tag="pt")
                nc.scalar.activation(
                    pt[:], pss[:], mybir.ActivationFunctionType.Exp, scale=SCALE
                )
                dp = kp - 2 * j  # diagonal pair index (0 or 1) if >= 0
                if dp >= 0:
                    nc.vector.tensor_mul(pt[:], pt[:], maskp[:, dp, :])
                for half in range(2):
                    kt = 2 * kp + half
                    nc.tensor.matmul(
                        psy[:, hh * TT : (hh + 1) * TT],
                        v_sb[:, kt, hh * HD : (hh + 1) * HD],
                        pt[:, half * TT : (half + 1) * TT],
                        start=(kp == 0 and half == 0),
                        stop=(kp == nkp - 1 and half == 1),
                    )
                if kp == 0:
                    nc.vector.tensor_copy(lacc[:], pt[:])
                else:
                    nc.vector.tensor_add(lacc[:], lacc[:], pt[:])

            def attn_tail(j, hh, psy, lacc):
                # denominator: ones-matmul partition-sum -> reciprocal ->
                # PE broadcast -> normalize. All fast units, no gpsimd.
                dps = pp.tile([P, 2 * TT], F32, tag="ps", name=f"dps{j}_{hh}")
                nc.tensor.matmul(
                    dps[0:1, 0:TT], ones_col[:], lacc[:, 0:TT],
                    start=True, stop=False,
                )
                nc.tensor.matmul(
                    dps[0:1, 0:TT], ones_col[:], lacc[:, TT : 2 * TT],
                    start=False, stop=True,
                )
                rinv1 = ryp.tile([1, TT], F32, tag="rinv")
                nc.vector.reciprocal_approx_fast(rinv1[:], dps[0:1, 0:TT])
                rinv1b = ryp.tile([1, TT], BF16, tag="rinvb")
                nc.vector.tensor_copy(rinv1b[:], rinv1[:])
                nc.tensor.matmul(
                    dps[:, TT : 2 * TT], ones_row[:], rinv1b[:],
                    start=True, stop=True,
                )
                rb = lrp.tile([P, TT], BF16, tag="lrep")
                nc.scalar.copy(rb[:], dps[:, TT : 2 * TT])
                yt = ytp.tile([P, TT], BF16, tag="yt")
                nc.vector.tensor_mul(yt[:], psy[:, hh * TT : (hh + 1) * TT], rb[:])
                yts[hh][j] = yt

            def oproj_unit(jj, s):
                # output rows t0..t0+127 ; 2 psum banks at a time (4 e-halves)
                ob = obp.tile([P, D], BF16, tag="ob")
                for eh in range(2):
                    pso = pp.tile([P, 2 * TT], F32, tag="ps", name=f"pso{jj}_{s}_{eh}")
                    for e2 in range(2):
                        e = 2 * eh + e2
                        for hh in range(HPC):
                            nc.tensor.matmul(
                                pso[:, e2 * TT : (e2 + 1) * TT],
                                yts[hh][jj][:, s * P : (s + 1) * P],
                                wo_sb[:, hh, e * TT : (e + 1) * TT],
                                start=(hh == 0),
                                stop=(hh == HPC - 1),
                            )
                    if eh == 0:
                        nc.vector.tensor_copy(ob[:, 0 : 2 * TT], pso[:])
                    else:
                        nc.scalar.copy(ob[:, 2 * TT : 4 * TT], pso[:])
                t0 = jj * TT + s * P
                nc.sync.dma_start(h["out"][t0 : t0 + P, :], ob[:])

            def make_proj_units(j):
                units = []
                psq = pp.tile([P, 2 * TT], F32, tag="ps", name=f"psq{j}")
                for c0 in (0, 4, 8, 12):
                    units.append(
                        lambda j=j, psq=psq, c0=c0: proj_chunk(j, wq_sb, psq, c0)
                    )
                units.append(lambda j=j, psq=psq: rope(j, psq, qs, qp, "qq"))
                psk = pp.tile([P, 2 * TT], F32, tag="ps", name=f"psk{j}")
                for c0 in (0, 4, 8, 12):
                    units.append(
                        lambda j=j, psk=psk, c0=c0: proj_chunk(j, wk_sb, psk, c0)
                    )
                units.append(lambda j=j, psk=psk: rope(j, psk, ks, kkp, "kk"))
                for half in range(2):
                    units.append(lambda j=j, half=half: proj_v(j, half))
                return units

            # ---- pipelined emission -----------------------------------------
            # prologue: x for t-tiles 0-1, projections for t-tile 0
            load_xj(0)
            load_xj(1)
            for u in make_proj_units(0):
                u()

            for j in range(NT):
                if j + 2 < NT:
                    load_xj(j + 2)

                proj_units = make_proj_units(j + 1) if j + 1 < NT else []

                att_units = []
                nkp = 2 * j + 2
                psy = pp.tile([P, 2 * TT], F32, tag="ps", name=f"psy{j}")
                laccs = [
                    lap.tile([P, 2 * TT], BF16, tag="lacc", name=f"lacc{j}_{_h}")
                    for _h in range(HPC)
                ]
                for kp in range(nkp):
                    for hh in range(HPC):
                        att_units.append(
                            lambda j=j, hh=hh, kp=kp, psy=psy, l=laccs, n=nkp:
                            attn_unit(j, hh, kp, psy, l[hh], n)
                        )
                for hh in range(HPC):
                    att_units.append(
                        lambda j=j, hh=hh, psy=psy, l=laccs:
                        attn_tail(j, hh, psy, l[hh])
                    )
                if j >= 2:
                    for s in range(4):
                        att_units.append(lambda jj=j - 2, s=s: oproj_unit(jj, s))
                if j == NT - 1:
                    for s in range(4):
                        att_units.append(lambda jj=j - 1, s=s: oproj_unit(jj, s))

                # Round-robin the two streams so the PE queue stays dense.
                na, np_ = len(att_units), len(proj_units)
                ia = ip = 0
                while ia < na or ip < np_:
                    if ip * max(na, 1) <= ia * max(np_, 1):
                        if ip < np_:
                            proj_units[ip]()
                            ip += 1
                        else:
                            att_units[ia]()
                            ia += 1
                    else:
                        if ia < na:
                            att_units[ia]()
                            ia += 1
                        else:
                            proj_units[ip]()
                            ip += 1

            for s in range(4):
                oproj_unit(NT - 1, s)

        if DEBUG:
            nc.sync.dma_start(h["dbg_q"][:], qs[0][:, 0:TT])
            nc.sync.dma_start(h["dbg_k"][:], ks[0][:, 0:TT])
            nc.sync.dma_start(h["dbg_v"][:], v_sb[:, 0, :])
            nc.sync.dma_start(h["dbg_y"][:], yts[0][0][:])


_CACHE = {}


def _program():
    if "nc" in _CACHE:
        return _CACHE["nc"]
    nc = bacc.Bacc(trn_type="TRN2")
    h = {
        "xt": nc.dram_tensor("xt", [D, T], F32, kind="ExternalInput"),
        "wq": nc.dram_tensor("wq", [D, DCORE], BF16, kind="ExternalInput"),
        "wk": nc.dram_tensor("wk", [D, DCORE], BF16, kind="ExternalInput"),
        "wv": nc.dram_tensor("wv", [D, DCORE], BF16, kind="ExternalInput"),
        "wo": nc.dram_tensor("wo", [DCORE, D], BF16, kind="ExternalInput"),
        "cos": nc.dram_tensor("cos", [P, T], BF16, kind="ExternalInput"),
        "sin": nc.dram_tensor("sin", [P, T], BF16, kind="ExternalInput"),
        "maskp": nc.dram_tensor("maskp", [P, 2 * 2 * TT], BF16, kind="ExternalInput"),
        "out": nc.dram_tensor("out", [T, D], BF16, kind="ExternalOutput"),
    }
    if DEBUG:
        h["dbg_q"] = nc.dram_tensor("dbg_q", [P, TT], BF16, kind="ExternalOutput")
        h["dbg_k"] = nc.dram_tensor("dbg_k", [P, TT], BF16, kind="ExternalOutput")
        h["dbg_v"] = nc.dram_tensor("dbg_v", [P, DCORE], BF16, kind="ExternalOutput")
        h["dbg_y"] = nc.dram_tensor("dbg_y", [P, TT], BF16, kind="ExternalOutput")
    with tile.TileContext(nc) as tc:
        _emit(nc, tc, h)
    nc.compile()
    _CACHE["nc"] = nc
    return nc


def _bf16(a):
    return np.asarray(a, dtype=np.float32).astype(ml_dtypes.bfloat16)


def _host_inputs(x, Wq, Wk, Wv, Wo):
    x = np.asarray(x, dtype=np.float32)
    xT = np.ascontiguousarray(x.reshape(T, D).T)  # [D, T]

    # rope tables, de-interleaved (evens then odds) with sign baked into sin
    inv = 1.0 / (ROPE_BASE ** (np.arange(0, HD, 2, dtype=np.float32) / HD))
    t = np.arange(T, dtype=np.float32)
    freqs = t[:, None] * inv[None, :]  # [T, 64]
    emb = np.concatenate([freqs, freqs], axis=-1)  # [T, 128]
    cos = np.cos(emb)
    sin = np.sin(emb)
    perm = np.concatenate([np.arange(0, HD, 2), np.arange(1, HD, 2)])
    cos_d = np.ascontiguousarray(cos[:, perm].T)  # [128, T]
    sgn = np.concatenate([-np.ones(64), np.ones(64)]).astype(np.float32)
    sin_d = np.ascontiguousarray(sgn[:, None] * sin[:, perm].T)

    # pair masks: maskp[k, dp*1024 + h*512 + q] = 1 iff q >= k + (2*dp+h)*128
    kk = np.arange(P)[:, None]
    qq = np.arange(TT)[None, :]
    mp = np.zeros((P, 2, 2, TT), dtype=np.float32)
    for dp in range(2):
        for hf in range(2):
            mp[:, dp, hf, :] = (qq >= kk + (2 * dp + hf) * P).astype(np.float32)
    mp = mp.reshape(P, 2 * 2 * TT)

    maps = []
    for i in range(NCORES):
        rows = np.concatenate(
            [(2 * i + hh) * HD + perm for hh in range(HPC)]
        )  # de-interleaved q/k rows for this core's heads
        vrows = np.arange(i * DCORE, (i + 1) * DCORE)
        maps.append(
            {
                "xt": xT,
                "wq": _bf16(np.asarray(Wq, np.float32)[rows, :].T),
                "wk": _bf16(np.asarray(Wk, np.float32)[rows, :].T),
                "wv": _bf16(np.asarray(Wv, np.float32)[vrows, :].T),
                "wo": _bf16(np.asarray(Wo, np.float32)[:, vrows].T),
                "cos": _bf16(cos_d),
                "sin": _bf16(sin_d),
                "maskp": _bf16(mp),
            }
        )
    return maps


def _run(x, Wq, Wk, Wv, Wo, trace=False):
    nc = _program()
    maps = _host_inputs(x, Wq, Wk, Wv, Wo)
    kw = {}
    if trace:
        kw = {"trace": True, "trace_cores": [0]}
    res = bass_utils.run_bass_kernel_spmd(
        nc, maps, core_ids=list(range(NCORES)), **kw
    )
    acc = np.zeros((T, D), dtype=np.float32)
    for r in res.results:
        acc += np.asarray(r["out"]).astype(np.float32)
    return acc.reshape(B, T, D), res


def kernel(x, Wq, Wk, Wv, Wo):
    out, _ = _run(x, Wq, Wk, Wv, Wo, trace=False)
    return out
